# revision 2
# baseline (speedup 1.0000x reference)
"""Trainium2 Bass kernel for nn_GCN1 (2-layer GCN + MLP head), v3.

vs baseline (686us): prop1's per-edge messages are laid out host-side as a
linear round-major stream (input replication only) and consumed with plain
HWDGE DMA + per-round DVE adds; prop2's dma_gather descriptors are
PRE-GENERATED (prepare_only on 4 SWDGE queues, one 8192-token tile per
queue ring) during prop1/AllGather and fired with trigger_dma right after
the AllGather lands.  Leaky chains are single ACT Lrelu ops; head PSUM
chain issues in descending block order to overlap prop2's add tail.
"""

import numpy as np

N = 15828
NP = 16384
S = 2048
SJ = S // 128
B = 64
NCORES = 8
HID = 100
TILE = 8192        # ring capacity: one 8192-token gather per SWDGE queue
ZROW = S - 1
NGROUPS = 5

NEG = 0.01
LA = (1.0 + NEG) / 2.0
LB = (1.0 - NEG) / 2.0


def _balance_nodes(deg_in):
    order = np.argsort(-deg_in, kind="stable")
    new_label = np.empty(N, dtype=np.int64)
    pos = np.zeros(NCORES, dtype=np.int64)
    edges = np.zeros(NCORES, dtype=np.int64)
    for start in range(0, N, NCORES):
        blk = order[start:start + NCORES]
        bins = np.argsort(edges, kind="stable")
        for i, n in enumerate(blk):
            k = int(bins[i])
            new_label[n] = k * S + pos[k]
            pos[k] += 1
            edges[k] += deg_in[n]
    assert pos.max() <= ZROW, pos
    return new_label


def _group_rounds(C, ngroups):
    R = len(C)
    INF = float("inf")
    dp = [[INF] * (R + 1) for _ in range(ngroups + 1)]
    par = [[0] * (R + 1) for _ in range(ngroups + 1)]
    dp[0][0] = 0
    for g in range(1, ngroups + 1):
        for j in range(1, R + 1):
            for i in range(j):
                if dp[g - 1][i] is INF:
                    continue
                c = dp[g - 1][i] + (j - i) * C[i]
                if c < dp[g][j]:
                    dp[g][j] = c
                    par[g][j] = i
    g = min(range(1, ngroups + 1), key=lambda g: dp[g][R])
    bounds = []
    j = R
    while g > 0:
        i = par[g][j]
        bounds.append((i, j))
        j = i
        g -= 1
        if j == 0:
            break
    return bounds[::-1]


def _idx_layout(v, cols):
    a = np.asarray(v, dtype=np.int16).reshape(cols, 16).T
    return np.tile(a, (NCORES, 1)).copy()


def _prep(in_feat, edge_index, W0, b0, W1, b1, lw0, lb0, lw2, lb2, lw3, lb3):
    assert not np.asarray(b0).any(), "kernel assumes b0 == 0 (GCN collapse)"
    src = np.asarray(edge_index[0], dtype=np.int64)
    dst = np.asarray(edge_index[1], dtype=np.int64)

    deg_out = np.maximum(np.bincount(src, minlength=N), 1)
    deg_in = np.maximum(np.bincount(dst, minlength=N), 1)

    new_label = _balance_nodes(np.bincount(dst, minlength=N).copy())
    src_n = new_label[src]
    dst_n = new_label[dst]

    w0 = np.asarray(W0, dtype=np.float64).reshape(-1)
    w1 = np.asarray(W1, dtype=np.float64).reshape(-1)
    alpha = LA * float(np.dot(w0, w1))
    beta = LB * float(np.dot(np.abs(w0), w1))
    apb = alpha + beta
    gamma = (alpha - beta) / apb
    b1f = float(np.asarray(b1).reshape(-1)[0])

    dego = np.ones(NP, dtype=np.float64)
    dego[new_label] = deg_out
    degi = np.ones(NP, dtype=np.float64)
    degi[new_label] = deg_in

    xs0n = np.zeros((NP + 1, B), dtype=np.float32)
    xs0n[new_label] = (np.asarray(in_feat, dtype=np.float32)[:, :, 0]
                       * (dego[new_label, None] ** -0.5).astype(np.float32))

    f1 = (degi * dego) ** -0.5
    fA = (alpha * f1).astype(np.float32)
    fB = (beta * f1).astype(np.float32)
    dis = (degi ** -0.5).astype(np.float32)
    fpacks = []
    for k in range(NCORES):
        fp = np.empty((128, 3 * SJ), dtype=np.float32)
        fp[:, :SJ] = fA[k * S:(k + 1) * S].reshape(SJ, 128).T
        fp[:, SJ:2 * SJ] = fB[k * S:(k + 1) * S].reshape(SJ, 128).T
        fp[:, 2 * SJ:] = dis[k * S:(k + 1) * S].reshape(SJ, 128).T
        fpacks.append(fp)

    csr = []
    for k in range(NCORES):
        m = (dst_n // S) == k
        dk = dst_n[m] - k * S
        sk = src_n[m]
        o = np.argsort(dk, kind="stable")
        dk, sk = dk[o], sk[o]
        indptr = np.zeros(S + 1, dtype=np.int64)
        np.add.at(indptr, dk + 1, 1)
        indptr = np.cumsum(indptr)
        csr.append((indptr, sk))

    degs_local = [np.diff(c[0]) for c in csr]
    maxdeg = int(max(d.max() for d in degs_local))
    Mhat = [max(int((d > j).sum()) for d in degs_local) for j in range(maxdeg)]
    C = [-(-m // 128) for m in Mhat]

    gb = _group_rounds(C, NGROUPS)
    groups = []
    rowbase = 0
    for (j0, j1) in gb:
        W = C[j0]
        groups.append((j0, j1 - j0, W, rowbase))
        rowbase += (j1 - j0) * W
    total_rows = rowbase
    e_pad = total_rows * 128
    icols = e_pad // 16

    def make_tok_src(k):
        indptr, sk = csr[k]
        d = degs_local[k]
        tk = np.full(total_rows * 128, NP, dtype=np.int64)
        for (j0, nr, W, rb) in groups:
            for jj in range(nr):
                j = j0 + jj
                cnt = int((d > j).sum())
                if cnt:
                    base = (rb + jj * W) * 128
                    tk[base:base + cnt] = sk[indptr[np.arange(cnt)] + j]
        return tk.reshape(total_rows, 128)

    msg1s, gidxs = [], []
    for k in range(NCORES):
        tk = make_tok_src(k)
        vals = xs0n[tk]                       # [rows, 128, 64]
        msg1s.append(np.ascontiguousarray(vals.transpose(1, 0, 2))
                     .reshape(128, total_rows * B))
        t = tk.reshape(-1)
        gidxs.append(_idx_layout(np.where(t == NP, 2047, t), icols))

    # tiles: runs of whole round-rows, <= TILE tokens
    tiles = []
    cur_rows = []
    cur_start = 0

    def flush(cs, cr):
        return (cs * 128, (cr[-1][0] + cr[-1][1] - cs) * 128, tuple(cr))

    for (j0, nr, W, rb) in groups:
        for jj in range(nr):
            r0 = rb + jj * W
            if cur_rows and (r0 + W - cur_start) * 128 > TILE:
                tiles.append(flush(cur_start, cur_rows))
                cur_rows = []
            if not cur_rows:
                cur_start = r0
            cur_rows.append((r0, W))
    if cur_rows:
        tiles.append(flush(cur_start, cur_rows))

    lw0n = np.zeros((HID, NP), dtype=np.float32)
    lw0n[:, new_label] = np.asarray(lw0, dtype=np.float32)
    lw0Ts = []
    for k in range(NCORES):
        blk = lw0n[:, k * S:(k + 1) * S].T
        blk = blk.reshape(SJ, 128, HID).transpose(1, 0, 2).reshape(128, SJ * HID)
        lw0Ts.append(np.ascontiguousarray(blk))
    lw2T = np.zeros((128, HID), dtype=np.float32)
    lw2T[:HID] = np.asarray(lw2, dtype=np.float32).T
    lw3T = np.zeros((128, 16), dtype=np.float32)
    lw3T[:HID, :10] = np.asarray(lw3, dtype=np.float32).T
    lbias = np.zeros((128, 4), dtype=np.float32)
    lbias[:HID, 0] = np.asarray(lb0, dtype=np.float32)
    lbias[:HID, 1] = np.asarray(lb2, dtype=np.float32)
    lbias[:10, 2] = np.asarray(lb3, dtype=np.float32)

    in_maps = []
    for k in range(NCORES):
        in_maps.append({
            "msg1": msg1s[k],
            "gidx": gidxs[k],
            "fpack": fpacks[k],
            "lbias": lbias,
            "lw0T": lw0Ts[k],
            "lw2T": lw2T,
            "lw3T": lw3T,
        })
    plan = (total_rows, tuple(groups), tuple(tiles), float(gamma), float(b1f))
    return in_maps, plan


def _build(plan):
    import concourse.bacc as bacc
    import concourse.mybir as mybir
    import concourse.tile as tile

    total_rows, groups, tiles, gamma, b1f = plan
    f32 = mybir.dt.float32
    i16 = mybir.dt.int16
    AL = mybir.AluOpType
    ACT = mybir.ActivationFunctionType
    e_pad = total_rows * 128
    icols = e_pad // 16
    NQ = 4

    nc = bacc.Bacc("TRN2", target_bir_lowering=False, debug=False,
                   num_devices=NCORES, num_swdge_queues=NQ)

    msg1_d = nc.dram_tensor("msg1", [128, total_rows * B], f32,
                            kind="ExternalInput")
    gidx_d = nc.dram_tensor("gidx", [128, icols], i16, kind="ExternalInput")
    fpack_d = nc.dram_tensor("fpack", [128, 3 * SJ], f32, kind="ExternalInput")
    lbias_d = nc.dram_tensor("lbias", [128, 4], f32, kind="ExternalInput")
    lw0T_d = nc.dram_tensor("lw0T", [128, SJ * HID], f32, kind="ExternalInput")
    lw2T_d = nc.dram_tensor("lw2T", [128, HID], f32, kind="ExternalInput")
    lw3T_d = nc.dram_tensor("lw3T", [128, 16], f32, kind="ExternalInput")
    out_d = nc.dram_tensor("out", [10, B], f32, kind="ExternalOutput")

    y1in_d = nc.dram_tensor("y1in", [S, B], f32)
    y1full_d = nc.dram_tensor("y1full", [NP, B], f32, addr_space="Shared")
    hpin_d = nc.dram_tensor("hpin", [HID, B], f32)
    hpout_d = nc.dram_tensor("hpout", [HID, B], f32, addr_space="Shared")

    grp = [list(range(NCORES))]

    with tile.TileContext(nc, trace_sim=False) as tc:
        with (
            tc.tile_pool(name="const", bufs=1) as cpool,
            tc.tile_pool(name="m1", bufs=3) as m1pool,
            tc.tile_pool(name="msg", bufs=1) as mpool,
            tc.tile_pool(name="psum", bufs=1, space="PSUM") as ppool,
        ):
            gix = cpool.tile([128, icols], i16)
            nc.sync.dma_start(gix[:], gidx_d.ap())
            fpk = cpool.tile([128, 3 * SJ], f32)
            nc.sync.dma_start(fpk[:], fpack_d.ap())
            lb_sb = cpool.tile([128, 4], f32)
            nc.sync.dma_start(lb_sb[:], lbias_d.ap())
            lw0T_sb = cpool.tile([128, SJ * HID], f32)
            nc.sync.dma_start(lw0T_sb[:], lw0T_d.ap())
            lw2T_sb = cpool.tile([128, HID], f32)
            nc.sync.dma_start(lw2T_sb[:], lw2T_d.ap())
            lw3T_sb = cpool.tile([128, 16], f32)
            nc.sync.dma_start(lw3T_sb[:], lw3T_d.ap())

            # ---- prop2 descriptor pre-generation: one tile per queue ring;
            # table read deps defer to the triggers (Tile handles this) ----
            dma_sems = [nc.alloc_semaphore(f"gsem{q}") for q in range(NQ)]
            msgs = []
            for t, (tok0, ntok, adds) in enumerate(tiles):
                msg = mpool.tile([128, TILE // 128, B], f32, tag=f"mg{t % NQ}")
                msgs.append(msg)
                if t < NQ:
                    nc.gpsimd.dma_gather(
                        msg[:, :ntok // 128, :], y1full_d.ap(),
                        gix[:, tok0 // 16:(tok0 + ntok) // 16],
                        ntok, ntok, B, prepare_only=True, sem=dma_sems[t],
                        queue_num=t, single_packet=False)

            # ---- prop1: linear HWDGE stream + per-round DVE adds ----
            a0 = cpool.tile([128, SJ, B], f32)
            nc.vector.memset(a0[:], 0.0)
            for (tok0, ntok, adds) in tiles:
                r0 = tok0 // 128
                nrows = ntok // 128
                m1 = m1pool.tile([128, TILE // 128, B], f32, tag="m1")
                nc.sync.dma_start(
                    m1[:, :nrows, :],
                    msg1_d.ap()[:, r0 * B:(r0 + nrows) * B]
                    .rearrange("p (r m) -> p r m", m=B))
                for (rr, W) in adds:
                    o = rr - r0
                    nc.vector.tensor_tensor(
                        a0[:, :W, :], a0[:, :W, :],
                        m1[:, o:o + W, :], AL.add)

            # ---- y1 = fA*a0 + fB*|a0| -> DRAM -> AllGather ----
            # (ACT Lrelu alpha is not honored on HW; use the explicit form)
            y1 = cpool.tile([128, SJ, B], f32)
            tmp = cpool.tile([128, SJ, B], f32)
            nc.scalar.activation(tmp[:], a0[:], ACT.Abs)
            nc.vector.tensor_tensor(
                tmp[:], tmp[:],
                fpk[:, SJ:2 * SJ].unsqueeze(2).broadcast_to([128, SJ, B]),
                AL.mult)
            nc.vector.tensor_tensor(
                y1[:], a0[:],
                fpk[:, :SJ].unsqueeze(2).broadcast_to([128, SJ, B]), AL.mult)
            nc.vector.tensor_tensor(y1[:], y1[:], tmp[:], AL.add)
            nc.sync.dma_start(y1in_d.ap().rearrange("(j p) m -> p j m", p=128),
                              y1[:])
            nc.gpsimd.collective_compute(
                "AllGather", AL.bypass, replica_groups=grp,
                ins=[y1in_d.ap().opt()], outs=[y1full_d.ap().opt()])

            # ---- fire the pre-generated gathers; late tiles gather normally
            for q in range(min(NQ, len(tiles))):
                ntok_q = tiles[q][1]
                # y1full dummy slice: WAW edge orders the trigger after the
                # AllGather that fills the gather table
                nc.gpsimd.trigger_dma(
                    count=None, queue_num=q,
                    signals_writable=[msgs[q][:, :ntok_q // 128, :],
                                      y1full_d.ap()[0:1, 0:1]])
            for t, (tok0, ntok, adds) in enumerate(tiles):
                if t >= NQ:
                    nc.gpsimd.dma_gather(
                        msgs[t][:, :ntok // 128, :], y1full_d.ap(),
                        gix[:, tok0 // 16:(tok0 + ntok) // 16],
                        ntok, ntok, B, queue_num=t % NQ, single_packet=False)

            a1 = cpool.tile([128, SJ, B], f32)
            nc.vector.memset(a1[:], 0.0)
            for t, (tok0, ntok, adds) in enumerate(tiles):
                base = tok0 // 128
                for (rr, W) in adds:
                    o = rr - base
                    add = nc.vector.tensor_tensor(
                        a1[:, :W, :], a1[:, :W, :],
                        msgs[t][:, o:o + W, :], AL.add)
                    if t < NQ:
                        # manual DMA sem: prepare_only drains signal gsem{t},
                        # which Tile's auto-sem pass doesn't know about
                        add._wait_ge(dma_sems[t], 16)

            # ---- h1 per block (descending) + head PSUM chain ----
            ps = ppool.tile([HID, B], f32)
            h1 = cpool.tile([128, SJ, B], f32)
            for j in reversed(range(SJ)):
                nc.vector.tensor_tensor(
                    h1[:, j, :], a1[:, j, :],
                    fpk[:, 2 * SJ + j:2 * SJ + j + 1].broadcast_to([128, B]),
                    AL.mult)
                nc.scalar.activation(h1[:, j, :], h1[:, j, :], ACT.Lrelu,
                                     bias=float(b1f), alpha=float(NEG))
                nc.tensor.matmul(ps[:], lhsT=lw0T_sb[:, j * HID:(j + 1) * HID],
                                 rhs=h1[:, j, :], start=(j == SJ - 1),
                                 stop=(j == 0))
            hp = cpool.tile([HID, B], f32)
            nc.vector.tensor_copy(hp[:], ps[:])
            nc.sync.dma_start(hpin_d.ap(), hp[:])
            nc.gpsimd.collective_compute(
                "AllReduce", AL.add, replica_groups=grp,
                ins=[hpin_d.ap().opt()], outs=[hpout_d.ap().opt()])

            z0 = cpool.tile([HID, B], f32)
            nc.sync.dma_start(z0[:], hpout_d.ap())
            nc.scalar.activation(z0[:], z0[:], ACT.Lrelu,
                                 bias=lb_sb[:HID, 0:1], alpha=float(NEG))
            ps2 = ppool.tile([HID, B], f32)
            nc.tensor.matmul(ps2[:], lhsT=lw2T_sb[:HID, :], rhs=z0[:],
                             start=True, stop=True)
            z1 = cpool.tile([HID, B], f32)
            nc.scalar.activation(z1[:], ps2[:], ACT.Lrelu,
                                 bias=lb_sb[:HID, 1:2], alpha=float(NEG))
            ps3 = ppool.tile([10, B], f32)
            nc.tensor.matmul(ps3[:], lhsT=lw3T_sb[:HID, 0:10], rhs=z1[:],
                             start=True, stop=True)
            z2 = cpool.tile([10, B], f32)
            nc.scalar.activation(z2[:], ps3[:], ACT.Lrelu,
                                 bias=lb_sb[:10, 2:3], alpha=float(NEG))
            nc.sync.dma_start(out_d.ap(), z2[:])

    nc.compile()
    return nc


_BUILD_CACHE = {}
LAST_RESULTS = None
RUN_KWARGS = {}


def kernel(**inputs) -> np.ndarray:
    global LAST_RESULTS
    from concourse.bass_utils import run_bass_kernel_spmd

    in_maps, plan = _prep(**inputs)
    if plan not in _BUILD_CACHE:
        _BUILD_CACHE[plan] = _build(plan)
    nc = _BUILD_CACHE[plan]

    res = run_bass_kernel_spmd(nc, in_maps, core_ids=list(range(NCORES)),
                               **RUN_KWARGS)
    LAST_RESULTS = res
    out = res.results[0]["out"]
    return np.ascontiguousarray(out.T.astype(np.float32))


# revision 3
# speedup vs baseline: 1.0241x; 1.0241x over previous
"""Trainium2 Bass kernel for nn_GCN1 (2-layer GCN + MLP head), v3.

vs baseline (686us): prop1's per-edge messages are laid out host-side as a
linear round-major stream (input replication only) and consumed with plain
HWDGE DMA + per-round DVE adds; prop2's dma_gather descriptors are
PRE-GENERATED (prepare_only on 4 SWDGE queues, one 8192-token tile per
queue ring) during prop1/AllGather and fired with trigger_dma right after
the AllGather lands.  Leaky chains are single ACT Lrelu ops; head PSUM
chain issues in descending block order to overlap prop2's add tail.
"""

import numpy as np

N = 15828
NP = 16384
S = 2048
SJ = S // 128
B = 64
NCORES = 8
HID = 100
TILE = 8192        # ring capacity: one 8192-token gather per SWDGE queue
ZROW = S - 1
NGROUPS = 5

NEG = 0.01
LA = (1.0 + NEG) / 2.0
LB = (1.0 - NEG) / 2.0


def _balance_nodes(deg_in):
    order = np.argsort(-deg_in, kind="stable")
    new_label = np.empty(N, dtype=np.int64)
    pos = np.zeros(NCORES, dtype=np.int64)
    edges = np.zeros(NCORES, dtype=np.int64)
    for start in range(0, N, NCORES):
        blk = order[start:start + NCORES]
        bins = np.argsort(edges, kind="stable")
        for i, n in enumerate(blk):
            k = int(bins[i])
            new_label[n] = k * S + pos[k]
            pos[k] += 1
            edges[k] += deg_in[n]
    assert pos.max() <= ZROW, pos
    return new_label


def _group_rounds(C, ngroups):
    R = len(C)
    INF = float("inf")
    dp = [[INF] * (R + 1) for _ in range(ngroups + 1)]
    par = [[0] * (R + 1) for _ in range(ngroups + 1)]
    dp[0][0] = 0
    for g in range(1, ngroups + 1):
        for j in range(1, R + 1):
            for i in range(j):
                if dp[g - 1][i] is INF:
                    continue
                c = dp[g - 1][i] + (j - i) * C[i]
                if c < dp[g][j]:
                    dp[g][j] = c
                    par[g][j] = i
    g = min(range(1, ngroups + 1), key=lambda g: dp[g][R])
    bounds = []
    j = R
    while g > 0:
        i = par[g][j]
        bounds.append((i, j))
        j = i
        g -= 1
        if j == 0:
            break
    return bounds[::-1]


def _idx_layout(v, cols):
    a = np.asarray(v, dtype=np.int16).reshape(cols, 16).T
    return np.tile(a, (NCORES, 1)).copy()


def _prep(in_feat, edge_index, W0, b0, W1, b1, lw0, lb0, lw2, lb2, lw3, lb3):
    assert not np.asarray(b0).any(), "kernel assumes b0 == 0 (GCN collapse)"
    src = np.asarray(edge_index[0], dtype=np.int64)
    dst = np.asarray(edge_index[1], dtype=np.int64)

    deg_out = np.maximum(np.bincount(src, minlength=N), 1)
    deg_in = np.maximum(np.bincount(dst, minlength=N), 1)

    new_label = _balance_nodes(np.bincount(dst, minlength=N).copy())
    src_n = new_label[src]
    dst_n = new_label[dst]

    w0 = np.asarray(W0, dtype=np.float64).reshape(-1)
    w1 = np.asarray(W1, dtype=np.float64).reshape(-1)
    alpha = LA * float(np.dot(w0, w1))
    beta = LB * float(np.dot(np.abs(w0), w1))
    apb = alpha + beta
    gamma = (alpha - beta) / apb
    b1f = float(np.asarray(b1).reshape(-1)[0])

    dego = np.ones(NP, dtype=np.float64)
    dego[new_label] = deg_out
    degi = np.ones(NP, dtype=np.float64)
    degi[new_label] = deg_in

    xs0n = np.zeros((NP + 1, B), dtype=np.float32)
    xs0n[new_label] = (np.asarray(in_feat, dtype=np.float32)[:, :, 0]
                       * (dego[new_label, None] ** -0.5).astype(np.float32))

    f1 = (degi * dego) ** -0.5
    fA = (alpha * f1).astype(np.float32)
    fB = (beta * f1).astype(np.float32)
    dis = (degi ** -0.5).astype(np.float32)
    fpacks = []
    for k in range(NCORES):
        fp = np.empty((128, 3 * SJ), dtype=np.float32)
        fp[:, :SJ] = fA[k * S:(k + 1) * S].reshape(SJ, 128).T
        fp[:, SJ:2 * SJ] = fB[k * S:(k + 1) * S].reshape(SJ, 128).T
        fp[:, 2 * SJ:] = dis[k * S:(k + 1) * S].reshape(SJ, 128).T
        fpacks.append(fp)

    csr = []
    for k in range(NCORES):
        m = (dst_n // S) == k
        dk = dst_n[m] - k * S
        sk = src_n[m]
        o = np.argsort(dk, kind="stable")
        dk, sk = dk[o], sk[o]
        indptr = np.zeros(S + 1, dtype=np.int64)
        np.add.at(indptr, dk + 1, 1)
        indptr = np.cumsum(indptr)
        csr.append((indptr, sk))

    degs_local = [np.diff(c[0]) for c in csr]
    maxdeg = int(max(d.max() for d in degs_local))
    Mhat = [max(int((d > j).sum()) for d in degs_local) for j in range(maxdeg)]
    C = [-(-m // 128) for m in Mhat]

    gb = _group_rounds(C, NGROUPS)
    groups = []
    rowbase = 0
    for (j0, j1) in gb:
        W = C[j0]
        groups.append((j0, j1 - j0, W, rowbase))
        rowbase += (j1 - j0) * W
    total_rows = rowbase
    e_pad = total_rows * 128
    icols = e_pad // 16

    def make_tok_src(k):
        indptr, sk = csr[k]
        d = degs_local[k]
        tk = np.full(total_rows * 128, NP, dtype=np.int64)
        for (j0, nr, W, rb) in groups:
            for jj in range(nr):
                j = j0 + jj
                cnt = int((d > j).sum())
                if cnt:
                    base = (rb + jj * W) * 128
                    tk[base:base + cnt] = sk[indptr[np.arange(cnt)] + j]
        return tk.reshape(total_rows, 128)

    msg1s, gidxs = [], []
    for k in range(NCORES):
        tk = make_tok_src(k)
        vals = xs0n[tk]                       # [rows, 128, 64]
        msg1s.append(np.ascontiguousarray(vals.transpose(1, 0, 2))
                     .reshape(128, total_rows * B))
        t = tk.reshape(-1)
        gidxs.append(_idx_layout(np.where(t == NP, 2047, t), icols))

    # tiles: runs of whole round-rows, <= TILE tokens
    tiles = []
    cur_rows = []
    cur_start = 0

    def flush(cs, cr):
        return (cs * 128, (cr[-1][0] + cr[-1][1] - cs) * 128, tuple(cr))

    for (j0, nr, W, rb) in groups:
        for jj in range(nr):
            r0 = rb + jj * W
            if cur_rows and (r0 + W - cur_start) * 128 > TILE:
                tiles.append(flush(cur_start, cur_rows))
                cur_rows = []
            if not cur_rows:
                cur_start = r0
            cur_rows.append((r0, W))
    if cur_rows:
        tiles.append(flush(cur_start, cur_rows))

    lw0n = np.zeros((HID, NP), dtype=np.float32)
    lw0n[:, new_label] = np.asarray(lw0, dtype=np.float32)
    lw0Ts = []
    for k in range(NCORES):
        blk = lw0n[:, k * S:(k + 1) * S].T
        blk = blk.reshape(SJ, 128, HID).transpose(1, 0, 2).reshape(128, SJ * HID)
        lw0Ts.append(np.ascontiguousarray(blk))
    lw2T = np.zeros((128, HID), dtype=np.float32)
    lw2T[:HID] = np.asarray(lw2, dtype=np.float32).T
    lw3T = np.zeros((128, 16), dtype=np.float32)
    lw3T[:HID, :10] = np.asarray(lw3, dtype=np.float32).T
    lbias = np.zeros((128, 4), dtype=np.float32)
    lbias[:HID, 0] = np.asarray(lb0, dtype=np.float32)
    lbias[:HID, 1] = np.asarray(lb2, dtype=np.float32)
    lbias[:10, 2] = np.asarray(lb3, dtype=np.float32)

    in_maps = []
    for k in range(NCORES):
        in_maps.append({
            "msg1": msg1s[k],
            "gidx": gidxs[k],
            "fpack": fpacks[k],
            "lbias": lbias,
            "lw0T": lw0Ts[k],
            "lw2T": lw2T,
            "lw3T": lw3T,
        })
    plan = (total_rows, tuple(groups), tuple(tiles), float(gamma), float(b1f))
    return in_maps, plan


def _build(plan):
    import concourse.bacc as bacc
    import concourse.mybir as mybir
    import concourse.tile as tile

    total_rows, groups, tiles, gamma, b1f = plan
    f32 = mybir.dt.float32
    i16 = mybir.dt.int16
    AL = mybir.AluOpType
    ACT = mybir.ActivationFunctionType
    e_pad = total_rows * 128
    icols = e_pad // 16
    NQ = 4

    nc = bacc.Bacc("TRN2", target_bir_lowering=False, debug=False,
                   num_devices=NCORES, num_swdge_queues=NQ,
                   dynamic_dma_scratch_size=32768)

    msg1_d = nc.dram_tensor("msg1", [128, total_rows * B], f32,
                            kind="ExternalInput")
    gidx_d = nc.dram_tensor("gidx", [128, icols], i16, kind="ExternalInput")
    fpack_d = nc.dram_tensor("fpack", [128, 3 * SJ], f32, kind="ExternalInput")
    lbias_d = nc.dram_tensor("lbias", [128, 4], f32, kind="ExternalInput")
    lw0T_d = nc.dram_tensor("lw0T", [128, SJ * HID], f32, kind="ExternalInput")
    lw2T_d = nc.dram_tensor("lw2T", [128, HID], f32, kind="ExternalInput")
    lw3T_d = nc.dram_tensor("lw3T", [128, 16], f32, kind="ExternalInput")
    out_d = nc.dram_tensor("out", [10, B], f32, kind="ExternalOutput")

    y1in_d = nc.dram_tensor("y1in", [S, B], f32)
    y1full_d = nc.dram_tensor("y1full", [NP, B], f32, addr_space="Shared")
    hpin_d = nc.dram_tensor("hpin", [HID, B], f32)
    hpout_d = nc.dram_tensor("hpout", [HID, B], f32, addr_space="Shared")

    grp = [list(range(NCORES))]

    with tile.TileContext(nc, trace_sim=False) as tc:
        with (
            tc.tile_pool(name="const", bufs=1) as cpool,
            tc.tile_pool(name="m1", bufs=2) as m1pool,
            tc.tile_pool(name="msg", bufs=1) as mpool,
            tc.tile_pool(name="psum", bufs=1, space="PSUM") as ppool,
        ):
            gix = cpool.tile([128, icols], i16)
            nc.sync.dma_start(gix[:], gidx_d.ap())
            fpk = cpool.tile([128, 3 * SJ], f32)
            nc.sync.dma_start(fpk[:], fpack_d.ap())
            lb_sb = cpool.tile([128, 4], f32)
            nc.sync.dma_start(lb_sb[:], lbias_d.ap())
            lw0T_sb = cpool.tile([128, SJ * HID], f32)
            nc.sync.dma_start(lw0T_sb[:], lw0T_d.ap())
            lw2T_sb = cpool.tile([128, HID], f32)
            nc.sync.dma_start(lw2T_sb[:], lw2T_d.ap())
            lw3T_sb = cpool.tile([128, 16], f32)
            nc.sync.dma_start(lw3T_sb[:], lw3T_d.ap())

            # ---- prop2 descriptor pre-generation.  All 5 tiles are prepped
            # onto queues 1-3 (2048-desc rings hold two tiles); queue 0 is
            # left unused and a tiny dummy gather absorbs the "first SWDGE
            # op runs inline on the Pool engine" penalty.  Table read deps
            # defer to the triggers (Tile handles this). ----
            dummy = cpool.tile([128, 1, B], f32)
            nc.gpsimd.dma_gather(
                dummy[:], msg1_d.ap().rearrange("p (r m) -> (p r) m", m=B),
                gix[:, 0:8], 128, 128, B, queue_num=0, single_packet=False)

            # tile -> queue; per-tile DMA sems (two gathers on one ring
            # interleave their engine increments, so per-queue sems race)
            tq = [1, 2, 3, 1, 2]
            dma_sems = [nc.alloc_semaphore(f"gsem{t}") for t in range(len(tiles))]
            msgs = []
            for t, (tok0, ntok, adds) in enumerate(tiles):
                msg = mpool.tile([128, TILE // 128, B], f32, tag=f"mg{t}")
                msgs.append(msg)
                nc.gpsimd.dma_gather(
                    msg[:, :ntok // 128, :], y1full_d.ap(),
                    gix[:, tok0 // 16:(tok0 + ntok) // 16],
                    ntok, ntok, B, prepare_only=True, sem=dma_sems[t],
                    queue_num=tq[t], single_packet=False)

            # ---- prop1: linear HWDGE stream + per-round DVE adds ----
            a0 = cpool.tile([128, SJ, B], f32)
            nc.vector.memset(a0[:], 0.0)
            for (tok0, ntok, adds) in tiles:
                r0 = tok0 // 128
                nrows = ntok // 128
                m1 = m1pool.tile([128, TILE // 128, B], f32, tag="m1")
                nc.sync.dma_start(
                    m1[:, :nrows, :],
                    msg1_d.ap()[:, r0 * B:(r0 + nrows) * B]
                    .rearrange("p (r m) -> p r m", m=B))
                for (rr, W) in adds:
                    o = rr - r0
                    nc.vector.tensor_tensor(
                        a0[:, :W, :], a0[:, :W, :],
                        m1[:, o:o + W, :], AL.add)

            # ---- y1 = fA*a0 + fB*|a0| -> DRAM -> AllGather ----
            # (ACT Lrelu alpha is not honored on HW; use the explicit form)
            y1 = cpool.tile([128, SJ, B], f32)
            tmp = cpool.tile([128, SJ, B], f32)
            nc.scalar.activation(tmp[:], a0[:], ACT.Abs)
            nc.vector.tensor_tensor(
                tmp[:], tmp[:],
                fpk[:, SJ:2 * SJ].unsqueeze(2).broadcast_to([128, SJ, B]),
                AL.mult)
            nc.vector.tensor_tensor(
                y1[:], a0[:],
                fpk[:, :SJ].unsqueeze(2).broadcast_to([128, SJ, B]), AL.mult)
            nc.vector.tensor_tensor(y1[:], y1[:], tmp[:], AL.add)
            nc.sync.dma_start(y1in_d.ap().rearrange("(j p) m -> p j m", p=128),
                              y1[:])
            nc.gpsimd.collective_compute(
                "AllGather", AL.bypass, replica_groups=grp,
                ins=[y1in_d.ap().opt()], outs=[y1full_d.ap().opt()])

            # ---- fire the pre-generated gathers (y1full dummy slice: WAW
            # edge orders each trigger after the AllGather) ----
            for q in (1, 2, 3):
                sw = [msgs[t][:, :tiles[t][1] // 128, :]
                      for t in range(len(tiles)) if tq[t] == q]
                nc.gpsimd.trigger_dma(
                    count=None, queue_num=q,
                    signals_writable=sw + [y1full_d.ap()[0:1, 0:1]])

            a1 = cpool.tile([128, SJ, B], f32)
            nc.vector.memset(a1[:], 0.0)
            for t, (tok0, ntok, adds) in enumerate(tiles):
                base = tok0 // 128
                for (rr, W) in adds:
                    o = rr - base
                    add = nc.vector.tensor_tensor(
                        a1[:, :W, :], a1[:, :W, :],
                        msgs[t][:, o:o + W, :], AL.add)
                    # manual DMA sem: prepare_only drains signal gsem{t},
                    # which Tile's auto-sem pass doesn't know about
                    add._wait_ge(dma_sems[t], 16)

            # ---- h1 per block (descending) + head PSUM chain ----
            ps = ppool.tile([HID, B], f32)
            h1 = cpool.tile([128, SJ, B], f32)
            for j in reversed(range(SJ)):
                nc.vector.tensor_tensor(
                    h1[:, j, :], a1[:, j, :],
                    fpk[:, 2 * SJ + j:2 * SJ + j + 1].broadcast_to([128, B]),
                    AL.mult)
                nc.scalar.activation(h1[:, j, :], h1[:, j, :], ACT.Lrelu,
                                     bias=float(b1f), alpha=float(NEG))
                nc.tensor.matmul(ps[:], lhsT=lw0T_sb[:, j * HID:(j + 1) * HID],
                                 rhs=h1[:, j, :], start=(j == SJ - 1),
                                 stop=(j == 0))
            hp = cpool.tile([HID, B], f32)
            nc.vector.tensor_copy(hp[:], ps[:])
            nc.sync.dma_start(hpin_d.ap(), hp[:])
            nc.gpsimd.collective_compute(
                "AllReduce", AL.add, replica_groups=grp,
                ins=[hpin_d.ap().opt()], outs=[hpout_d.ap().opt()])

            z0 = cpool.tile([HID, B], f32)
            nc.sync.dma_start(z0[:], hpout_d.ap())
            nc.scalar.activation(z0[:], z0[:], ACT.Lrelu,
                                 bias=lb_sb[:HID, 0:1], alpha=float(NEG))
            ps2 = ppool.tile([HID, B], f32)
            nc.tensor.matmul(ps2[:], lhsT=lw2T_sb[:HID, :], rhs=z0[:],
                             start=True, stop=True)
            z1 = cpool.tile([HID, B], f32)
            nc.scalar.activation(z1[:], ps2[:], ACT.Lrelu,
                                 bias=lb_sb[:HID, 1:2], alpha=float(NEG))
            ps3 = ppool.tile([10, B], f32)
            nc.tensor.matmul(ps3[:], lhsT=lw3T_sb[:HID, 0:10], rhs=z1[:],
                             start=True, stop=True)
            z2 = cpool.tile([10, B], f32)
            nc.scalar.activation(z2[:], ps3[:], ACT.Lrelu,
                                 bias=lb_sb[:10, 2:3], alpha=float(NEG))
            nc.sync.dma_start(out_d.ap(), z2[:])

    nc.compile()
    return nc


_BUILD_CACHE = {}
LAST_RESULTS = None
RUN_KWARGS = {}


def kernel(**inputs) -> np.ndarray:
    global LAST_RESULTS
    from concourse.bass_utils import run_bass_kernel_spmd

    in_maps, plan = _prep(**inputs)
    if plan not in _BUILD_CACHE:
        _BUILD_CACHE[plan] = _build(plan)
    nc = _BUILD_CACHE[plan]

    res = run_bass_kernel_spmd(nc, in_maps, core_ids=list(range(NCORES)),
                               **RUN_KWARGS)
    LAST_RESULTS = res
    out = res.results[0]["out"]
    return np.ascontiguousarray(out.T.astype(np.float32))
